# revision 29
# baseline (speedup 1.0000x reference)
"""KNN classification kernel for Trainium2 (8 NeuronCores).

Problem: B=1024 queries x N=200000 gallery, D=256, top-10 neighbors,
softmax-weighted one-hot class scores over 50 classes.

Math fold: reference computes gallery = l2norm(train.T, axis=1) -- i.e. each
feature dim d is normalized by ||train[:, d]|| over the FULL gallery. That
scale folds into the query side, so the device computes a pure matmul plus a
top-k screen; the host reranks screened spans exactly in f64.

Device (per core, gallery sharded along N into 8 x 25000, zero-padded to
25088 = 49 x 512), all in fp8-e4m3 with DoubleRow perf mode (K=256 folded
into one PE instruction at 0.5 cycles/row).  Sims live in PSUM and can only
leave through DVE or Act, so the screen is split so neither engine is the
bottleneck; per 128-query batch, the 24.5 1024-col chunks go to two lanes
with independent double-buffered PSUM tags (so the chains don't serialize):
  DVE lane (11 chunks):    top-8 values per chunk (InstMax)
  Act lane  (13.5 chunks): bf16 copy to SBUF, then DMA to DRAM; the host
                           screens those sims directly (bf16 error ~8 device
                           units << margin)
Host: tau_q = 10th-largest device sim among all witnesses (DVE top-8s and
  dumped bf16 sims); exactly rerank (f64) every span whose max-witness >=
  tau - mu (mu = 600 device units ~ 16 sigmas of the measured fp8 matmul
  noise, far beyond its bounded worst case); exact top-10 -> softmax ->
  class scores, identical math to the reference.
"""

import os
import numpy as np

NB_KNN = 10
T = 0.07
NUM_CLASSES = 50
EPS = 1e-12

B, N, D = 1024, 200000, 256
NCORES = 8
NPC = N // NCORES          # 25000 real cols per core
TILE = 512
NPC_PAD = 25088            # 49 * 512
CHUNK = 1024               # consumer chunk width
NCH = 24                   # full 1024-chunks per core (+ one 512 tail)
# gallery DMA blocks as (tile_start, ntiles): the 512 tail loads first (DVE
# screens it during warmup), then small lead-in blocks so Act starts early
BLOCKS = [(48, 1), (0, 4), (4, 4), (8, 8), (16, 8), (24, 8), (32, 8), (40, 8)]

SQ = 64.0                  # query fp8 pre-scale
SG = 16.0                  # gallery fp8 pre-scale
MU = 600.0                 # fp8-noise screen margin, device units

# chunk lanes per 4-chunk gallery block: A,A,D,D x5 blocks, then A,A,A,D,
# then the 512 tail -> D.  A pairs share one dump DMA.
_LANES = (["A", "D", "A", "D"] * 2 + ["A", "A", "A", "D"]
          + ["A", "D", "A", "D"] * 3)
D_CHUNKS = [i for i, l in enumerate(_LANES) if l == "D"]   # 11 chunks
A_CHUNKS = [i for i, l in enumerate(_LANES) if l == "A"]   # 13 chunks
ND = len(D_CHUNKS) + 1       # + the 512 tail chunk
NA = len(A_CHUNKS)
ARAW_W = NA * CHUNK          # 13312 bf16 sims dumped per (query, core)

_CACHE = {}


def _build_bass():
    import concourse.bacc as bacc
    import concourse.tile as tile
    from concourse import mybir

    nc = bacc.Bacc("TRN2")
    f8 = mybir.dt.float8e4
    bf16 = mybir.dt.bfloat16
    f32 = mybir.dt.float32
    DR = mybir.MatmulPerfMode.DoubleRow
    Copy = mybir.ActivationFunctionType.Copy

    g_d = nc.dram_tensor("g", [2, 128, NPC_PAD], f8, kind="ExternalInput")
    q_d = nc.dram_tensor("q", [2, 128, B], f8, kind="ExternalInput")
    cand_d = nc.dram_tensor("cand", [B, ND * 8], f32, kind="ExternalOutput")
    araw_d = nc.dram_tensor("araw", [B, ARAW_W], bf16, kind="ExternalOutput")

    with tile.TileContext(nc) as tc:
        with tc.tile_pool(name="qp", bufs=1) as qp, \
             tc.tile_pool(name="gp", bufs=2) as gp, \
             tc.tile_pool(name="cp", bufs=8) as cp, \
             tc.tile_pool(name="sp", bufs=4) as sp, \
             tc.tile_pool(name="pp", bufs=1, space="PSUM") as pp:
            q8 = qp.tile([128, 2, B], f8, tag="q8")
            nc.sync.dma_start(out=q8[:],
                              in_=q_d[:].rearrange("a p b -> p a b"))

            cands = [cp.tile([128, ND * 8], f32, tag="cand",
                             name=f"cand{i}") for i in range(8)]

            def consume(bc, ci, w, lhs, g8, goff, pend, last=False,
                        scan_w=None):
                """Emit matmuls for chunk ci (width w) and its consumer."""
                lane = "D" if ci >= NCH else _LANES[ci]
                ps = pp.tile([128, CHUNK], f32, tag=f"ps{lane}", bufs=2,
                             name=f"ps{lane}_{bc}_{ci}")
                for s in range(w // TILE):
                    nc.tensor.matmul(
                        ps[:, s * TILE:(s + 1) * TILE], lhs,
                        g8[:, :, goff + s * TILE:goff + (s + 1) * TILE],
                        start=True, stop=True, perf_mode=DR)
                if lane == "D":
                    k = ND - 1 if ci >= NCH else D_CHUNKS.index(ci)
                    nc.vector.max(cands[bc][:, k * 8:(k + 1) * 8],
                                  ps[:, :(scan_w or w)])
                    return pend
                # Act lane: bf16 copy, buffer into a paired scratch, dump
                # one DMA per completed pair (or at a lone/tail chunk).
                ai = A_CHUNKS.index(ci)
                if pend is None:
                    pend = (sp.tile([128, 2 * CHUNK], bf16, tag="scr",
                                    bufs=8, name=f"scr_{bc}_{ci}"), [])
                scr, slots = pend
                half = len(slots)
                nc.scalar.activation(out=scr[:, half * CHUNK:half * CHUNK + w],
                                     in_=ps[:, :w], func=Copy)
                slots.append((ai, w))
                # flush on a full pair, or when the next chunk is not an A
                # continuation (block end / lane switch)
                nxt_a = ci + 1 < NCH and _LANES[ci + 1] == "A" and not last
                if len(slots) == 2 or not nxt_a:
                    a0, w0 = slots[0]
                    wtot = sum(wi for _, wi in slots)
                    nc.sync.dma_start(
                        out=araw_d[bc * 128:(bc + 1) * 128,
                                   a0 * CHUNK:a0 * CHUNK + wtot],
                        in_=scr[:, :wtot])
                    return None
                return pend

            for blk, (t0, ntile) in enumerate(BLOCKS):
                cw = ntile * TILE
                c0 = t0 * TILE
                g8 = gp.tile([128, 2, cw], f8, tag=f"g8_{blk}", bufs=1,
                             name=f"g8_{blk}")
                nc.sync.dma_start(
                    out=g8[:],
                    in_=g_d[:, :, c0:c0 + cw].rearrange("a p b -> p a b"))
                for bc in range(8):
                    lhs = q8[:, :, bc * 128:(bc + 1) * 128]
                    pend = None
                    if ntile == 1:                  # 512 tail -> DVE lane
                        pend = consume(bc, NCH, TILE, lhs, g8, 0, pend,
                                       scan_w=NPC - 48 * TILE)
                    else:
                        for j in range(ntile // 2):  # 1024-chunks
                            ci = (t0 * TILE) // CHUNK + j
                            pend = consume(bc, ci, CHUNK, lhs, g8,
                                           j * CHUNK, pend,
                                           last=(j == ntile // 2 - 1))
                            if ci == 19:             # most DVE slots done
                                nc.sync.dma_start(
                                    out=cand_d[bc * 128:(bc + 1) * 128, :72],
                                    in_=cands[bc][:, :72])
                            if ci == NCH - 1:        # bc's last DVE chunk
                                nc.sync.dma_start(
                                    out=cand_d[bc * 128:(bc + 1) * 128, 72:],
                                    in_=cands[bc][:, 72:])
                    assert pend is None
    if not nc.is_finalized():
        nc.finalize()
    return nc


def _run_device(g_shards, q_packed):
    from concourse.bass_utils import run_bass_kernel_spmd
    if "nc" not in _CACHE:
        _CACHE["nc"] = _build_bass()
    nc = _CACHE["nc"]
    in_maps = [{"g": g_shards[c], "q": q_packed} for c in range(NCORES)]
    res = run_bass_kernel_spmd(nc, in_maps, list(range(NCORES)))
    cand = np.concatenate(
        [res.results[c]["cand"] for c in range(NCORES)], axis=1)
    araw = np.stack([res.results[c]["araw"] for c in range(NCORES)], axis=1)
    return cand, araw                     # araw: [B, NCORES, ARAW_W] bf16


def _run_emulated(g_shards, q_packed):
    import ml_dtypes
    qf = q_packed.astype(np.float32).reshape(256, B)
    cands, araws = [], []
    for c in range(NCORES):
        gf = g_shards[c].astype(np.float32).reshape(256, NPC_PAD)
        sim = qf.T @ gf                                   # [B, NPC_PAD]
        cd = np.empty((B, ND * 8), np.float32)
        for k, ci in enumerate(D_CHUNKS):
            blkv = sim[:, ci * CHUNK:(ci + 1) * CHUNK]
            cd[:, k * 8:(k + 1) * 8] = -np.sort(-blkv, axis=1)[:, :8]
        tailv = sim[:, NCH * CHUNK:NCH * CHUNK + TILE]
        cd[:, (ND - 1) * 8:ND * 8] = -np.sort(-tailv, axis=1)[:, :8]
        ar = np.concatenate(
            [sim[:, ci * CHUNK:(ci + 1) * CHUNK] for ci in A_CHUNKS], axis=1)
        cands.append(cd)
        araws.append(ar.astype(ml_dtypes.bfloat16))
    return np.concatenate(cands, axis=1), np.stack(araws, axis=1)


def kernel(test_features, train_features, train_labels):
    import ml_dtypes
    f8 = ml_dtypes.float8_e4m3

    test_features = np.asarray(test_features, dtype=np.float32)
    train_features = np.asarray(train_features, dtype=np.float32)
    train_labels = np.asarray(train_labels)

    # ---- host pre: fold normalizations into the query side ----
    tf64 = train_features.astype(np.float64)
    norm_d = np.maximum(np.sqrt(np.sum(tf64 * tf64, axis=0)), EPS)
    q64 = test_features.astype(np.float64)
    qn = np.sqrt(np.sum(q64 * q64, axis=1, keepdims=True))
    q_scaled = q64 / np.maximum(qn, EPS) / norm_d          # [B, D] f64
    # unit-normalized device queries: same per-query ranking as q_scaled
    row = np.sqrt(np.sum(q_scaled * q_scaled, axis=1, keepdims=True))
    q_unit = q_scaled / np.maximum(row, EPS)

    q_packed = np.ascontiguousarray(
        (q_unit.T * SQ).astype(f8).reshape(2, 128, B))
    gt8 = (train_features.T * SG).astype(f8)               # [D, N]
    g_shards = []
    for c in range(NCORES):
        sl = np.zeros((256, NPC_PAD), dtype=f8)
        sl[:, :NPC] = gt8[:, c * NPC:(c + 1) * NPC]
        g_shards.append(np.ascontiguousarray(sl.reshape(2, 128, NPC_PAD)))

    # ---- device: fp8 matmul + two-lane screen ----
    if os.environ.get("KNN_EMULATE"):
        cand, araw = _run_emulated(g_shards, q_packed)
    else:
        cand, araw = _run_device(g_shards, q_packed)
    cand = cand.astype(np.float64)            # [B, NCORES*ND*8]
    araw = araw.astype(np.float32)            # [B, NCORES, ARAW_W]

    # ---- host post: screen -> exact f64 rerank -> softmax scores ----
    # per-query witness threshold: 10th-largest device sim seen
    # ARAW_W = 13*1024 = 26*512: screen dumped sims at 512-col granularity
    a_chmax = araw.reshape(B, NCORES, 26, 512).max(axis=3)     # [B,C,26]
    dve_top1 = cand.reshape(B, NCORES, ND, 8)[:, :, :, 0]      # [B,C,ND]
    wit = np.concatenate(
        [cand, a_chmax.reshape(B, -1)], axis=1)
    tau = -np.partition(-wit, NB_KNN - 1, axis=1)[:, NB_KNN - 1]
    thresh = tau - MU                                          # [B]

    # span table: DVE chunks (1024 cols) + Act half-chunks (512 cols)
    spans = []          # (col0, col1) global
    sel_cols = []       # [B] bool per span
    for c in range(NCORES):
        base = c * NPC
        for k, ci in enumerate(D_CHUNKS):
            spans.append((base + ci * CHUNK,
                          base + min((ci + 1) * CHUNK, NPC)))
            sel_cols.append(dve_top1[:, c, k] >= thresh)
        spans.append((base + NCH * CHUNK, base + NPC))      # 512 tail (DVE)
        sel_cols.append(dve_top1[:, c, ND - 1] >= thresh)
        for h in range(26):
            ci = A_CHUNKS[h // 2]
            c0 = base + ci * CHUNK + (h % 2) * 512
            spans.append((c0, min(c0 + 512, base + NPC)))
            sel_cols.append(a_chmax[:, c, h] >= thresh)
    sel = np.stack(sel_cols, axis=1)                       # [B, nspans]

    reg_queries = {}
    for b in range(B):
        for r in np.nonzero(sel[b])[0]:
            reg_queries.setdefault(int(r), []).append(b)

    per_q_vals = [[] for _ in range(B)]
    per_q_cols = [[] for _ in range(B)]
    for r, qs in reg_queries.items():
        c0, c1 = spans[r]
        if c0 >= c1:
            continue
        block = tf64[c0:c1]                                # [w, D] view
        sims = q_scaled[qs] @ block.T                      # [nq, w] f64
        cols = np.arange(c0, c1)
        for i, b in enumerate(qs):
            per_q_vals[b].append(sims[i])
            per_q_cols[b].append(cols)

    labels = train_labels.astype(np.int64)
    scores = np.zeros((B, NUM_CLASSES), dtype=np.float64)
    for b in range(B):
        v = np.concatenate(per_q_vals[b])
        cidx = np.concatenate(per_q_cols[b])
        sel_i = np.argpartition(-v, NB_KNN - 1)[:NB_KNN]
        order = np.lexsort((cidx[sel_i], -v[sel_i]))
        sel_i = sel_i[order]
        topv = v[sel_i]
        w = np.exp(topv / T - np.max(topv) / T)
        w /= w.sum()
        np.add.at(scores[b], labels[cidx[sel_i]], w)
    return scores.astype(np.float32)


if __name__ == "__main__":
    rng = np.random.default_rng(0)
    tf = rng.standard_normal((B, D), dtype=np.float32)
    trf = rng.standard_normal((N, D), dtype=np.float32)
    trl = rng.integers(0, NUM_CLASSES, N).astype(np.int64)
    os.environ["KNN_EMULATE"] = "1"
    out = kernel(tf, trf, trl)
    print(out.shape, out.dtype, out.sum())
